# revision 7
# baseline (speedup 1.0000x reference)
"""Trainium2 Bass kernel for nn_AutoencoderInverseAffine.

out[n] = (samples[n] - mus_[symb[n], comp[n]]) / psi_c[comp[n]] + mus_orig_[symb[n], comp[n]]
       = samples[n] * A[comp[n]] + B[symb[n]*4 + comp[n]]

with A = 1/psi (4x8) and B = mus_orig - mus * A (64x8) precomputed on host
(tiny parameter tables). The 8M rows are data-parallel across the 8
NeuronCores; on-device each row's two table vectors are materialized via
per-class is_equal masks + predicated copies on the Vector engine, then a
fused multiply-add produces the output.
"""

import os
import numpy as np

import concourse.bass as bass
import concourse.bacc as bacc
import concourse.mybir as mybir
import concourse.tile as tile
from concourse.bass_utils import run_bass_kernel_spmd
from contextlib import ExitStack

F32 = mybir.dt.float32
BF16 = mybir.dt.bfloat16
U32 = mybir.dt.uint32

N_SAMP = 8388608
N_DIM = 8
NX = 16
N_COMP = 4
N_CLASS = NX * N_COMP  # 64
NCORES = 8
R = N_SAMP // NCORES   # rows per core
C = 512                # rows per partition per tile
NT = R // (128 * C)    # tiles per core

_cache = {}


def _build():
    nc = bacc.Bacc("TRN2", target_bir_lowering=False, debug=False,
                   num_devices=NCORES)
    samp = nc.dram_tensor("samples", (R, N_DIM), BF16, kind="ExternalInput").ap()
    jidx = nc.dram_tensor("jidx", (R,), BF16, kind="ExternalInput").ap()
    cidx = nc.dram_tensor("cidx", (R,), BF16, kind="ExternalInput").ap()
    tabd = nc.dram_tensor("tab", (128, (N_COMP + N_CLASS) * N_DIM), BF16,
                          kind="ExternalInput").ap()
    outd = nc.dram_tensor("out", (R, N_DIM), BF16, kind="ExternalOutput").ap()

    s3 = samp.rearrange("(t p c) d -> t p (c d)", p=128, c=C)
    o3 = outd.rearrange("(t p c) d -> t p (c d)", p=128, c=C)
    j3 = jidx.rearrange("(t p c) -> t p c", p=128, c=C)
    c3 = cidx.rearrange("(t p c) -> t p c", p=128, c=C)

    with tile.TileContext(nc) as tc, ExitStack() as ctx:
        consts = ctx.enter_context(tc.tile_pool(name="consts", bufs=1))
        io = ctx.enter_context(tc.tile_pool(name="io", bufs=2))
        work = ctx.enter_context(tc.tile_pool(name="work", bufs=1))
        outp = ctx.enter_context(tc.tile_pool(name="outp", bufs=2))

        tab = consts.tile([128, (N_COMP + N_CLASS) * N_DIM], BF16)
        nc.gpsimd.dma_start(tab[:], tabd[:])

        def tab_vec(k):
            # class-k 8-bf16 vector, bitcast to 4 u32, broadcast to (128, C, 4)
            v = tab[:, 8 * k:8 * k + 8].bitcast(U32)
            return v.unsqueeze(1).broadcast_to([128, C, N_DIM // 2])

        for t in range(NT):
            st = io.tile([128, C * N_DIM], BF16, tag="samp")
            nc.gpsimd.dma_start(st[:], s3[t])
            jt = io.tile([128, C], BF16, tag="jidx")
            nc.gpsimd.dma_start(jt[:], j3[t])
            ct = io.tile([128, C], BF16, tag="cidx")
            nc.gpsimd.dma_start(ct[:], c3[t])

            gA = work.tile([128, C * N_DIM], BF16, tag="gA")
            gB = work.tile([128, C * N_DIM], BF16, tag="gB")
            mask = work.tile([128, C], mybir.dt.uint8, tag="mask")
            gA3 = gA[:].bitcast(U32).rearrange("p (c d) -> p c d", d=N_DIM // 2)
            gB3 = gB[:].bitcast(U32).rearrange("p (c d) -> p c d", d=N_DIM // 2)
            mask3 = mask[:].unsqueeze(2).broadcast_to([128, C, N_DIM // 2])

            for k in range(N_COMP):
                nc.vector.tensor_scalar(mask[:], ct[:], float(k), None,
                                        mybir.AluOpType.is_equal)
                nc.vector.copy_predicated(gA3, mask3, tab_vec(k))
            for k in range(N_CLASS):
                nc.vector.tensor_scalar(mask[:], jt[:], float(k), None,
                                        mybir.AluOpType.is_equal)
                nc.vector.copy_predicated(gB3, mask3, tab_vec(N_COMP + k))

            prod = work.tile([128, C * N_DIM], BF16, tag="prod")
            nc.vector.tensor_mul(prod[:], st[:], gA[:])
            ot = outp.tile([128, C * N_DIM], BF16, tag="out")
            nc.vector.tensor_add(ot[:], prod[:], gB[:])
            nc.gpsimd.dma_start(o3[t], ot[:])

    nc.compile()
    return nc


def _host_prep(samples_, mus_orig_, mus_, psi_c_, idx_symb_, idx_comp_):
    import ml_dtypes
    bf16 = ml_dtypes.bfloat16
    A = (1.0 / psi_c_.reshape(N_COMP, N_DIM)).astype(np.float32)
    mu3 = np.asarray(mus_).reshape(NX, N_COMP, N_DIM).astype(np.float32)
    mo3 = np.asarray(mus_orig_).reshape(NX, N_COMP, N_DIM).astype(np.float32)
    B = (mo3 - mu3 * A[None]).reshape(N_CLASS, N_DIM).astype(np.float32)
    tab1 = np.concatenate([A.reshape(-1), B.reshape(-1)]).astype(bf16)
    tab = np.broadcast_to(tab1, (128, tab1.size)).copy()

    j = (np.asarray(idx_symb_, dtype=np.int64) * N_COMP
         + np.asarray(idx_comp_, dtype=np.int64)).astype(bf16)
    c = np.asarray(idx_comp_, dtype=np.float32).astype(bf16)
    samples = np.ascontiguousarray(np.asarray(samples_, dtype=np.float32)).astype(bf16)
    return samples, j, c, tab


def kernel(samples_, mus_orig_, mus_, psi_c_, idx_symb_, idx_comp_,
           n_samp_=None, n_dim_=None, **_unused):
    samples, j, c, tab = _host_prep(np.asarray(samples_), np.asarray(mus_orig_),
                                    np.asarray(mus_), np.asarray(psi_c_),
                                    np.asarray(idx_symb_), np.asarray(idx_comp_))
    if "nc" not in _cache:
        _cache["nc"] = _build()
    nc = _cache["nc"]

    in_maps = []
    for i in range(NCORES):
        sl = slice(i * R, (i + 1) * R)
        in_maps.append({
            "samples": samples[sl],
            "jidx": j[sl],
            "cidx": c[sl],
            "tab": tab,
        })

    trace = bool(os.environ.get("KERNEL_TRACE"))
    kwargs = {}
    if trace:
        # antenv.axon_hooks is missing in this image; shim it so trace works.
        import sys
        import types
        if "antenv.axon_hooks" not in sys.modules:
            import trn_agent_boot.trn_boot as _tb
            m = types.ModuleType("antenv.axon_hooks")
            holder = [None]
            m.set_axon_ntff_profile_hook = lambda h: holder.__setitem__(0, h)
            m.get_axon_ntff_profile_hook = lambda: holder[0]
            sys.modules["antenv.axon_hooks"] = m
            m.set_axon_ntff_profile_hook(
                _tb._ntff_profile_via_ctypes("/opt/axon/libaxon_pjrt.so"))
        kwargs = {"trace": True,
                  "tmpdir": os.environ.get("KERNEL_TRACE_DIR") or None}

    res = run_bass_kernel_spmd(nc, in_maps, core_ids=list(range(NCORES)), **kwargs)
    if trace:
        _cache["exec_time_ns"] = res.exec_time_ns
        _cache["profile_json"] = res.profile_json

    out = np.concatenate([res.results[i]["out"] for i in range(NCORES)], axis=0)
    return out.astype(np.float32)
